# revision 20
# baseline (speedup 1.0000x reference)
"""Bass/Tile Trainium2 kernel for additive (Bahdanau/'cat') attention.

Problem (per batch b):
  A[i,d]      = sum_a context[i,a] * attn_w[a,d] + attn_b[d]
  O[o,d]      = sum_e output[o,e]  * dec_w[e,d]  + dec_b[d]
  scores[o,i] = sum_d query_w[d] * tanh(A[i,d] + O[o,d])   (+query_b: softmax-invariant)
  attn        = softmax_i(scores)
  mix[o,a]    = sum_i attn[o,i] * context[i,a]
  out[o,d]    = tanh([mix | output] @ out_w + out_b)

Sharding: pure data-parallel over batch, B=8 -> one batch per NeuronCore,
weights broadcast, no collectives.

Per-core structure (v6 — TB/X two-lane, ready-ordered emission):
  * The 16.7M-element nonlinearity is split between ACT and DVE per
    (o, d-chunk) unit with NO broadcast-add anywhere:
    - TB lane (ACT): the raw A^T matmul result stays RESIDENT IN PSUM and
      each unit is ONE activation: Fc[j] = tanh(pa_dc + bias) with
      bias = O^T[d,o] + dec_b + attn_b (biases folded at O^T evacuation).
      ~523 ns/unit uncontended, ~700 under DVE SBUF contention.
    - X lane (DVE): exp-domain form R' = 1/(1 + e^{2A'} e^{2O'}) computed
      by ONE fused custom DVE op per block (u = Src0*Src1 + 1, bitwise-not
      seed, one Newton pass; bf16 in/out; o-batched via stride-0 broadcast
      APs; ~0.92 cyc/elem).  tanh(x) = 1 - 2R', so the PE weight for X
      rows is -2*q_d (vs q_d for TB rows); the per-row constant sum q_d
      is softmax-invariant.
    Assignment JT=(16,JT1,0,0): dc0 all-TB, dc1 mixed (pa0/pa1 resident
    PSUM banks), dc2/dc3 all-X.
  * Engine FIFOs are strict in-order, so everything is emitted in DATA-
    READY order: per phase the PE gets [dc3-gA burst, dc0 chase with the
    dc3-gB burst spliced mid-way, dc1-X, dc2 half-bursts, dc1-TB chase];
    per-dc Fc pools (bufs=2) recycle independently so neither engine
    stalls at the phase boundary; epilogue A is emitted right after the
    first phase-B X op so its DVE tail isn't buried behind 38us of X ops.
  * All inputs PRE-ARRANGED PARTITION-MAJOR + bf16 on the host (one
    dma_start each, 4-8KB descriptors); DMA queues (sync/scalar HWDGE +
    gpsimd SWDGE) are load-balanced with the A^T/O^T inputs first.
  * Scores for group g accumulate at PSUM partitions 32g (tile_position);
    softmax exp reads PSUM directly (accum_out normalizer); attn^T packed
    per group; mix + final projection per phase-half, output^T/bias
    chunks of the projection pre-accumulated early.
"""

import numpy as np
import ml_dtypes

import concourse.bass as bass
import concourse.tile as tile
import concourse.bass_utils as bass_utils
from concourse import bacc, mybir, dve_ops
from concourse.dve_ops import DveOp
from concourse.dve_spec import Spec, Src0, Src1, AluOp, Bin, One, C0, C1
from concourse.masks import make_identity

B, OUT_LEN, IN_LEN, DEC, ATTN = 8, 64, 512, 512, 512
P = 128
F32 = mybir.dt.float32
BF16 = mybir.dt.bfloat16
AF = mybir.ActivationFunctionType

G = 16                    # o's per group (= PSUM col-group rows)
NG = OUT_LEN // G         # 4 groups
DC = DEC // P             # 4 d-chunks
AC = ATTN // P            # 4 a-chunks
IC = IN_LEN // P          # 4 i-chunks
EC = DEC // P             # 4 e-chunks
CC = (ATTN + DEC) // P    # 8 combined chunks

N_CORES = 8

JT1 = 14                  # TB rows in dc1 (dc0 = 16, dc2 = dc3 = 0)
JT = (16, JT1, 0, 0)

# seed + one-Newton reciprocal constants (Chebyshev pair for the
# [-4.5, -4] interval x*bitcast(~x) lands in; same as RECIP_APPROX_FAST).
C0V, C1V = -0.23549792, 2.0017324

# ---- fused DVE op: out = 1/(Src0*Src1 + 1) ---------------------------------
_u = Src0 * Src1 + One
_nu = Bin(AluOp.BITWISE_NOT, _u, _u)
_y0 = _nu * C0
_RECIP_BODY = _y0 * (C1 - _u * _y0)


def _recip_ref(in0, in1, c0, c1, c2):
    u = (in0.astype(np.float32) * in1.astype(np.float32) + np.float32(1.0)).astype(
        np.float32
    )
    nx = (~u.view(np.int32)).view(np.float32)
    y0 = nx * np.float32(c0)
    return (y0 * (np.float32(c1) - u * y0)).astype(np.float32)


RECIP_AFFINE1 = DveOp(
    "RECIP_AFFINE1_ANT",
    Spec(body=_RECIP_BODY, reference=_recip_ref),
    subdim=False,
    uops_sha={"v3": "4a6026d53837a2bc", "v4": "9de0b962752db8fb"},
)

if RECIP_AFFINE1.name not in dve_ops._SUB_OPCODE_FOR_NAME:
    dve_ops.OPS.append(RECIP_AFFINE1)
    dve_ops.CUSTOM_DVE_SPECS[RECIP_AFFINE1.name] = RECIP_AFFINE1.spec
    dve_ops._SUB_OPCODE_FOR_NAME[RECIP_AFFINE1.name] = (
        dve_ops._CUSTOM_DVE_ROW_BASE + len(dve_ops.OPS) - 1
    )


class _Ctx:
    """Shared tiles/state for the phase emitters."""


def _x_op(nc, C, Fc, dc, g, lo, hi):
    o0 = G * g
    n = hi - lo
    nc.vector._custom_dve(
        RECIP_AFFINE1,
        out=Fc[:, lo:hi, :],
        in0=C.Pexp[:, dc, :].unsqueeze(1).broadcast_to([P, n, IN_LEN]),
        in1=C.Qexp[:, dc, o0 + lo : o0 + hi].unsqueeze(2).broadcast_to(
            [P, n, IN_LEN]
        ),
        s0=C0V,
        s1=C1V,
    )


def _mm(nc, C, ps, Fc, dc, g, j, start=False, stop=False):
    nc.tensor.matmul(
        ps[32 * g : 32 * g + G, :],
        C.QZ[:, dc, j, :],
        Fc[:, j, :],
        start=start,
        stop=stop,
        tile_position=(0, 32 * g),
        skip_group_check=True,
    )


def _tb(nc, C, Fc, pa, dc, g, j):
    o = G * g + j
    nc.scalar.activation(
        Fc[:, j, :], pa[:], AF.Tanh, bias=C.OTb2[:, dc, o : o + 1]
    )


def _phase_xops_a(nc, C, gs):
    """Phase-A DVE X ops: coarse (merged dc3 pair-op, full per-group dc2)
    — the op-count overhead matters more than arrival granularity since
    phase A's matmul bursts hide under phase B."""
    g0, g1 = gs
    F = {}
    F["3m"] = C.fp[3].tile([P, 2 * G, IN_LEN], BF16, tag="F3", name="F_A_3m")
    o0 = G * g0
    nc.vector._custom_dve(
        RECIP_AFFINE1,
        out=F["3m"][:, :, :],
        in0=C.Pexp[:, 3, :].unsqueeze(1).broadcast_to([P, 2 * G, IN_LEN]),
        in1=C.Qexp[:, 3, o0 : o0 + 2 * G].unsqueeze(2).broadcast_to(
            [P, 2 * G, IN_LEN]
        ),
        s0=C0V,
        s1=C1V,
    )
    for g in gs:
        F[1, g] = C.fp[1].tile([P, G, IN_LEN], BF16, tag="F1", name=f"F_{g}_1")
        _x_op(nc, C, F[1, g], 1, g, JT1, G)
    for g in gs:
        F[2, g] = C.fp[2].tile([P, G, IN_LEN], BF16, tag="F2", name=f"F_{g}_2")
        _x_op(nc, C, F[2, g], 2, g, 0, G)
    return F


def _phase_main_a(nc, C, gs, F, ps):
    g0, g1 = gs
    # dc0: all-TB, matmuls chasing the singles
    F0 = {g: C.fp[0].tile([P, G, IN_LEN], BF16, tag="F0", name=f"F_{g}_0")
          for g in gs}
    for j in range(G):
        for g in gs:
            _tb(nc, C, F0[g], C.pa0, 0, g, j)
        for g in gs:
            _mm(nc, C, ps, F0[g], 0, g, j, start=(j == 0))
    # dc3 merged burst
    for j in range(G):
        for gl, g in enumerate(gs):
            nc.tensor.matmul(
                ps[32 * g : 32 * g + G, :], C.QZ[:, 3, j, :],
                F["3m"][:, G * gl + j, :],
                start=False, stop=False,
                tile_position=(0, 32 * g), skip_group_check=True,
            )
    # dc1 X rows
    for j in range(JT1, G):
        for g in gs:
            _mm(nc, C, ps, F[1, g], 1, g, j)
    # dc1: TB singles chased
    for j in range(JT1):
        for g in gs:
            _tb(nc, C, F[1, g], C.pa1, 1, g, j)
        for g in gs:
            _mm(nc, C, ps, F[1, g], 1, g, j)
    # dc2 bursts last; they carry the accumulation stops
    for g in gs:
        for j in range(G):
            _mm(nc, C, ps, F[2, g], 2, g, j, stop=(j == G - 1))


def _phase_xops(nc, C, gs):
    """Phase-B first DVE X op (dc3 of gs[0]) — fills the phase boundary."""
    F = {}
    F[3, gs[0]] = C.fp[3].tile([P, G, IN_LEN], BF16, tag="F3", name=f"F_{gs[0]}_3")
    _x_op(nc, C, F[3, gs[0]], 3, gs[0], 0, G)
    return F


def _phase_xops_rest(nc, C, gs, F):
    g1 = gs[1]
    F[3, g1] = C.fp[3].tile([P, G, IN_LEN], BF16, tag="F3", name=f"F_{g1}_3")
    _x_op(nc, C, F[3, g1], 3, g1, 0, G)
    for g in gs:
        F[1, g] = C.fp[1].tile([P, G, IN_LEN], BF16, tag="F1", name=f"F_{g}_1")
        _x_op(nc, C, F[1, g], 1, g, JT1, G)
    for g in gs:
        F[2, g] = C.fp[2].tile([P, G, IN_LEN], BF16, tag="F2", name=f"F_{g}_2")
    for half in range(2):
        for g in gs:
            _x_op(nc, C, F[2, g], 2, g, half * (G // 2), (half + 1) * (G // 2))


def _phase_main(nc, C, gs, F, ps):
    """ACT singles + all q-reduction matmuls for one phase, emitted in
    data-ready order so no engine FIFO head blocks."""
    g0, g1 = gs
    # dc3 gs[0] X data is ready before the first single: burst it first.
    for j in range(G):
        _mm(nc, C, ps, F[3, g0], 3, g0, j, start=(j == 0))
    # dc0: all-TB, matmuls chasing the singles; splice the dc3-g1 burst in
    # after 8 rows (its X op lands about then).
    F0 = {g: C.fp[0].tile([P, G, IN_LEN], BF16, tag="F0", name=f"F_{g}_0")
          for g in gs}
    for j in range(G):
        for g in gs:
            _tb(nc, C, F0[g], C.pa0, 0, g, j)
        for g in gs:
            _mm(nc, C, ps, F0[g], 0, g, j, start=(g == g1 and j == 0))
        if j == 8:
            for jj in range(G):
                _mm(nc, C, ps, F[3, g1], 3, g1, jj)
    # dc1 X rows
    for j in range(JT1, G):
        for g in gs:
            _mm(nc, C, ps, F[1, g], 1, g, j)
    # dc2 first halves
    for j in range(G // 2):
        for g in gs:
            _mm(nc, C, ps, F[2, g], 2, g, j)
    # dc1: TB singles chased
    for j in range(JT1):
        for g in gs:
            _tb(nc, C, F[1, g], C.pa1, 1, g, j)
        for g in gs:
            _mm(nc, C, ps, F[1, g], 1, g, j)
    # dc2 second halves last (their X data arrives last on the DVE);
    # they carry the accumulation stop.
    for j in range(G // 2, G):
        for g in gs:
            _mm(nc, C, ps, F[2, g], 2, g, j, stop=(j == G - 1))


def _epilogue_phase(nc, C, gs, ps, psum, attn_d):
    """softmax + attn^T + per-group mix for groups gs (one phase half).
    Staged so the critical chain (exp -> recip -> attn_bf -> transpose ->
    mix -> combT) never waits behind the attn-output muls/DMAs."""
    SL = {g: slice(32 * g, 32 * g + G) for g in gs}
    for g in gs:
        nc.scalar.activation(C.exp_sb[SL[g], :], ps[SL[g], :], AF.Exp,
                             accum_out=C.sums[SL[g], :])
    for g in gs:
        nc.vector.reciprocal(C.recip[SL[g], :], C.sums[SL[g], :])
        nc.vector.tensor_scalar_mul(C.attn_bf[SL[g], :], C.exp_sb[SL[g], :],
                                    C.recip[SL[g], :])
    for g in gs:
        sl = SL[g]
        for ic in range(IC):
            pt = psum.tile([P, G], BF16, tag="sm", bufs=2, name=f"pt_{g}_{ic}")
            nc.tensor.transpose(
                pt[:], C.attn_bf[sl, ic * P : (ic + 1) * P], C.ident_bf[sl, sl],
                tile_position=(32 * g, 0),
            )
            nc.vector.tensor_copy(C.attnT_bf[:, ic, G * g : G * g + G], pt[:])
    for g in gs:
        # per-group mix so the first group's matmuls overlap the second's
        cols = slice(G * g, G * g + G)
        for ac in range(AC):
            pm = psum.tile([P, G], F32, tag="sm", bufs=2, name=f"pm_{g}_{ac}")
            for ic in range(IC):
                nc.tensor.matmul(
                    pm[:],
                    C.ctx_bf[:, ic, ac * P : (ac + 1) * P],
                    C.attnT_bf[:, ic, cols],
                    start=(ic == 0),
                    stop=(ic == IC - 1),
                )
            nc.vector.tensor_copy(C.combT_bf[:, ac, cols], pm[:])
    # attn output (off the critical path)
    for g in gs:
        nc.vector.tensor_scalar_mul(C.attn_sb[SL[g], :], C.exp_sb[SL[g], :],
                                    C.recip[SL[g], :])
        nc.sync.dma_start(attn_d[G * g : G * g + G, :], C.attn_sb[SL[g], :])


def _project_pre(nc, C, ph):
    """output^T chunks + bias of the final projection for half ph — inputs
    are ready as soon as the DMAs land, so accumulate them early."""
    hsl = slice(32 * ph, 32 * ph + 32)
    pv = C.po_f[hsl, :]
    for k, cc in enumerate(range(EC, CC)):
        nc.tensor.matmul(
            pv, C.combT_bf[:, cc, hsl], C.out_w_bf[:, cc, :],
            start=(k == 0), stop=False, tile_position=(0, 32 * ph),
            skip_group_check=True,
        )
    nc.tensor.matmul(
        pv, C.ones_bf[:, 0:32], C.outb_row_bf[:], start=False, stop=False,
        tile_position=(0, 32 * ph), skip_group_check=True,
    )


def _project_rest(nc, C, ph, out_d):
    hsl = slice(32 * ph, 32 * ph + 32)
    pv = C.po_f[hsl, :]
    for cc in range(EC):
        nc.tensor.matmul(
            pv, C.combT_bf[:, cc, hsl], C.out_w_bf[:, cc, :],
            start=False, stop=(cc == EC - 1), tile_position=(0, 32 * ph),
            skip_group_check=True,
        )
    nc.scalar.activation(C.out_sb[hsl, :], pv, AF.Tanh)
    nc.sync.dma_start(out_d[hsl, :], C.out_sb[hsl, :])


def _build_body(tc):
    nc = tc.nc

    # ---- DRAM I/O (partition-major host layouts, bf16 weights) ----
    dt = nc.dram_tensor
    ctxT_d = dt("ctx_t_pm", [P, AC * IN_LEN], BF16, kind="ExternalInput").ap()
    ctx_d = dt("ctx_pm", [P, IC * ATTN], BF16, kind="ExternalInput").ap()
    outT_d = dt("out_t_pm", [P, EC * OUT_LEN], BF16, kind="ExternalInput").ap()
    attn_w_d = dt("attn_w_pm", [P, AC * DEC], BF16, kind="ExternalInput").ap()
    dec_w_d = dt("dec_w_pm", [P, EC * DEC], BF16, kind="ExternalInput").ap()
    out_w_d = dt("out_w_pm", [P, CC * DEC], BF16, kind="ExternalInput").ap()
    outb_row_d = dt("outb_row_bf", [1, DEC], BF16, kind="ExternalInput").ap()
    ab2_d = dt("ab2", [P, DC], F32, kind="ExternalInput").ap()
    adb_d = dt("adb", [P, DC], F32, kind="ExternalInput").ap()
    db2_d = dt("db2", [P, DC], F32, kind="ExternalInput").ap()
    qz_d = dt("qz", [P, DC * G * G], BF16, kind="ExternalInput").ap()
    out_d = dt("out", [OUT_LEN, DEC], F32, kind="ExternalOutput").ap()
    attn_d = dt("attn", [OUT_LEN, IN_LEN], F32, kind="ExternalOutput").ap()

    from contextlib import ExitStack

    with ExitStack() as ctx:
        const = ctx.enter_context(tc.tile_pool(name="const", bufs=1))
        statics = ctx.enter_context(tc.tile_pool(name="statics", bufs=1))
        psum = ctx.enter_context(tc.tile_pool(name="psum", bufs=1, space="PSUM"))

        C = _Ctx()
        # per-dc Fc pools: each recycles independently (A -> B)
        C.fp = [ctx.enter_context(tc.tile_pool(name=f"fp{d}", bufs=2))
                for d in range(DC)]

        # ---------------- constants / identity / warmup ----------------
        ident = const.tile([P, P], F32)
        make_identity(nc, ident)
        C.ident_bf = const.tile([P, P], BF16)
        nc.vector.tensor_copy(C.ident_bf[:], ident[:])

        wu = psum.tile([P, P], F32, tag="mm", bufs=1)
        for _ in range(10):
            nc.tensor.matmul(wu[:], C.ident_bf[:], C.ident_bf[:], start=True, stop=True)

        # ---------------- input DMAs (one per tensor, critical first) ----
        C.ctxT_bf = statics.tile([P, AC, IN_LEN], BF16)    # [a%, ac, i]
        C.attn_w_bf = statics.tile([P, AC, DEC], BF16)     # [a%, ac, d]
        C.dec_w_bf = statics.tile([P, EC, DEC], BF16)      # [e%, ec, d]
        C.ctx_bf = statics.tile([P, IC, ATTN], BF16)       # [i%, ic, a]
        C.out_w_bf = statics.tile([P, CC, DEC], BF16)      # [c%, cc, d]
        C.combT_bf = statics.tile([P, CC, OUT_LEN], BF16)  # [c%, cc, o]
        C.QZ = const.tile([P, DC, G, G], BF16)
        ab2 = const.tile([P, DC], F32)
        adb = const.tile([P, DC], F32)
        db2 = const.tile([P, DC], F32)
        C.outb_row_bf = const.tile([1, DEC], BF16)
        C.ones_bf = const.tile([1, OUT_LEN], BF16)
        nc.vector.memset(C.ones_bf[:], 1.0)

        r3 = lambda ap, n: ap.rearrange("p (c i) -> p c i", c=n)
        # A^T inputs split half/half across both HWDGE queues so each lands
        # in ~half the single-queue time; O^T inputs on the gpsimd SWDGE.
        nc.scalar.dma_start(
            C.QZ[:], qz_d.rearrange("p (dc j m) -> p dc j m", dc=DC, j=G)
        )
        for t, d in ((ab2, ab2_d), (adb, adb_d), (db2, db2_d)):
            nc.scalar.dma_start(t[:], d)
        nc.scalar.dma_start(C.outb_row_bf[:], outb_row_d)
        h = AC // 2
        nc.sync.dma_start(C.ctxT_bf[:, 0:h, :], r3(ctxT_d, AC)[:, 0:h, :])
        nc.scalar.dma_start(C.ctxT_bf[:, h:AC, :], r3(ctxT_d, AC)[:, h:AC, :])
        nc.sync.dma_start(C.attn_w_bf[:, 0:h, :], r3(attn_w_d, AC)[:, 0:h, :])
        nc.scalar.dma_start(C.attn_w_bf[:, h:AC, :], r3(attn_w_d, AC)[:, h:AC, :])
        nc.gpsimd.dma_start(C.dec_w_bf[:], r3(dec_w_d, EC))
        # output^T chunks double as combined^T chunks 4..7
        nc.gpsimd.dma_start(C.combT_bf[:, EC:CC, :], r3(outT_d, EC))
        nc.sync.dma_start(C.ctx_bf[:], r3(ctx_d, IC))
        nc.scalar.dma_start(C.out_w_bf[:], r3(out_w_d, CC))

        # ---------------- prologue: A^T, O^T, exps ----------------
        C.OTb2 = statics.tile([P, DC, OUT_LEN], F32)  # O^T + dec_b + attn_b
        C.Pexp = statics.tile([P, DC, IN_LEN], BF16)  # e^{2(A+attn_b)}
        C.Qexp = statics.tile([P, DC, OUT_LEN], BF16)  # e^{2(O+dec_b)}

        def _ot_dc(dc):
            po = psum.tile([P, OUT_LEN], F32, tag="sm", bufs=2, name=f"po_{dc}")
            for ec in range(EC):
                nc.tensor.matmul(
                    po[:],
                    C.dec_w_bf[:, ec, dc * P : (dc + 1) * P],
                    C.combT_bf[:, EC + ec, :],
                    start=(ec == 0),
                    stop=(ec == EC - 1),
                )
            nc.vector.tensor_scalar_add(C.OTb2[:, dc, :], po[:], adb[:, dc : dc + 1])
            if JT[dc] < G:
                nc.scalar.activation(
                    C.Qexp[:, dc, :], po[:], AF.Exp, bias=db2[:, dc : dc + 1],
                    scale=2.0,
                )

        def _at_dc(dc, pa):
            for ac in range(AC):
                nc.tensor.matmul(
                    pa[:],
                    C.attn_w_bf[:, ac, dc * P : (dc + 1) * P],
                    C.ctxT_bf[:, ac, :],
                    start=(ac == 0),
                    stop=(ac == AC - 1),
                )
            if JT[dc] < G:
                nc.scalar.activation(
                    C.Pexp[:, dc, :], pa[:], AF.Exp, bias=ab2[:, dc : dc + 1],
                    scale=2.0,
                )

        # ---------------- epilogue tiles ----------------
        C.exp_sb = statics.tile([P, IN_LEN], F32)
        C.sums = statics.tile([P, 1], F32)
        C.recip = statics.tile([P, 1], F32)
        C.attn_sb = statics.tile([P, IN_LEN], F32)
        C.attn_bf = statics.tile([P, IN_LEN], BF16)
        C.attnT_bf = statics.tile([P, IC, OUT_LEN], BF16)
        C.out_sb = statics.tile([OUT_LEN, DEC], F32)
        psA = psum.tile([P, IN_LEN], F32, tag="scA", bufs=1, name="psA")
        psB = psum.tile([P, IN_LEN], F32, tag="scB", bufs=1, name="psB")
        C.po_f = psum.tile([OUT_LEN, DEC], F32, tag="pj", bufs=1, name="po_f")
        C.pa0 = psum.tile([P, IN_LEN], F32, tag="pa0", bufs=1, name="pa0")
        C.pa1 = psum.tile([P, IN_LEN], F32, tag="pa1", bufs=1, name="pa1")

        # ---------------- prologue ----------------
        pa3 = psum.tile([P, IN_LEN], F32, tag="mm", bufs=1, name="pa3")
        _at_dc(3, pa3)
        _at_dc(0, C.pa0)
        _ot_dc(3)
        _ot_dc(0)
        pa2 = psum.tile([P, IN_LEN], F32, tag="mm", bufs=1, name="pa2")
        _at_dc(2, pa2)
        _ot_dc(2)
        _at_dc(1, C.pa1)
        _ot_dc(1)

        # ---------------- main loop ----------------
        FA = _phase_xops_a(nc, C, (0, 1))
        _phase_main_a(nc, C, (0, 1), FA, psA)
        _project_pre(nc, C, 0)
        FB = _phase_xops(nc, C, (2, 3))           # dc3-g2 X op
        _epilogue_phase(nc, C, (0, 1), psA, psum, attn_d)
        _phase_xops_rest(nc, C, (2, 3), FB)
        _phase_main(nc, C, (2, 3), FB, psB)
        _project_rest(nc, C, 0, out_d)
        _project_pre(nc, C, 1)
        _epilogue_phase(nc, C, (2, 3), psB, psum, attn_d)
        _project_rest(nc, C, 1, out_d)


_CACHE = {}


def build_nc():
    if "nc" in _CACHE:
        return _CACHE["nc"]
    nc = bacc.Bacc(
        "TRN2",
        target_bir_lowering=False,
        debug=False,
        num_devices=N_CORES,
    )
    with tile.TileContext(nc) as tc:
        _build_body(tc)
    nc.compile()
    _CACHE["nc"] = nc
    return nc


def _pm(a, nchunk):
    """[nchunk*128, F] -> partition-major [128, nchunk*F]."""
    n, f = a.shape
    assert n == nchunk * P
    return np.ascontiguousarray(
        a.reshape(nchunk, P, f).transpose(1, 0, 2).reshape(P, nchunk * f)
    )


def _shared_inputs(inputs):
    f32 = lambda k: np.ascontiguousarray(np.asarray(inputs[k], dtype=np.float32))
    bf = lambda a: np.ascontiguousarray(np.asarray(a, dtype=ml_dtypes.bfloat16))
    attn_b = f32("attn_b").reshape(ATTN)
    dec_b = f32("dec_b").reshape(DEC)
    q = f32("query_w").reshape(DEC)
    # [P, DC] per-partition layouts: t[p, dc] = v[dc*128 + p]
    pd = lambda v: np.ascontiguousarray(v.reshape(DC, P).T)
    q_pd = pd(q)
    qz = np.zeros((P, DC, G, G), np.float32)
    for dc in range(DC):
        for j in range(G):
            w = q_pd[:, dc] if j < JT[dc] else -2.0 * q_pd[:, dc]
            qz[:, dc, j, j] = w
    return {
        "attn_w_pm": _pm(bf(f32("attn_w")), AC),
        "dec_w_pm": _pm(bf(f32("dec_w")), EC),
        "out_w_pm": _pm(bf(f32("out_w")), CC),
        "outb_row_bf": bf(f32("out_b").reshape(1, DEC)),
        "ab2": pd(2.0 * attn_b),
        "adb": pd(attn_b + dec_b),
        "db2": pd(2.0 * dec_b),
        "qz": bf(qz.reshape(P, DC * G * G)),
    }


def make_core_inputs(inputs, b):
    """Per-core input map for batch b (host-side layout prep only)."""
    bf = lambda a: np.ascontiguousarray(np.asarray(a, dtype=ml_dtypes.bfloat16))
    m = dict(_shared_inputs(inputs))
    ctx = np.asarray(inputs["context"], np.float32)[b]
    out = np.asarray(inputs["output"], np.float32)[b]
    m["ctx_pm"] = _pm(bf(ctx), IC)
    m["ctx_t_pm"] = _pm(bf(ctx.T), AC)
    m["out_t_pm"] = _pm(bf(out.T), EC)
    return m


def kernel(**inputs):
    nc = build_nc()
    in_maps = [make_core_inputs(inputs, b) for b in range(N_CORES)]
    res = bass_utils.run_bass_kernel_spmd(nc, in_maps, core_ids=list(range(N_CORES)))
    _CACHE["last_results"] = res
    out = np.stack([res.results[b]["out"] for b in range(N_CORES)])
    attn = np.stack([res.results[b]["attn"] for b in range(N_CORES)])
    return out, attn
